# revision 2
# baseline (speedup 1.0000x reference)
"""TRN2 Bass kernel for batched dot-product attention (no scale, eval mode).

reference:
    score   = einsum('bqd,bvd->bqv', query, value)      # B=16, L=2048, D=1024
    attn    = softmax(score, axis=-1)
    context = einsum('bqv,bvd->bqd', attn, value)

Sharding: data-parallel over batch; each of 8 NeuronCores handles 2 batch
elements, no communication.

Per-core per-batch plan (all matmuls in float32r = full-rate TF32-like):
  - preload V: natural copy (f32r) for the PV matmul + PE-transposed copy VT
    (f32r) for the QK^T matmul
  - per 128-row q-tile:
      Q tile -> PE transpose -> QT [d,q] f32r
      S = QT.T @ VT chunks -> PSUM [128, 2048]
      rowmax (DVE, per 512-chunk) -> exp(S - max) on ACT with fused rowsum
      P -> PE transpose -> PT [v,q] f32r
      O = PT.T @ Vnat -> PSUM [128, 1024]; multiply by 1/rowsum; DMA out
"""

from contextlib import ExitStack

import numpy as np

import concourse.bass as bass
import concourse.tile as tile
from concourse import bacc, mybir
from concourse.bass_utils import run_bass_kernel_spmd
from concourse.masks import make_identity

B, LQ, LV, D = 16, 2048, 2048, 1024
NCORES = 8
BPC = B // NCORES  # batches per core
P = 128
NQT = LQ // P  # 16 q tiles
NVT = LV // P  # 16 v tiles
ND = D // P  # 8 d tiles
VCH = 512  # MM1 moving-operand chunk (free dim)
NCH = LV // VCH  # 4
DCH = 512  # MM2 moving-operand chunk
NDCH = D // DCH  # 2

f32 = mybir.dt.float32
f32r = mybir.dt.float32r
EXP = mybir.ActivationFunctionType.Exp
AX = mybir.AxisListType.X


def build_nc():
    nc = bacc.Bacc("TRN2", target_bir_lowering=False, debug=False)
    q_d = nc.dram_tensor("q", [BPC, LQ, D], f32, kind="ExternalInput").ap()
    v_d = nc.dram_tensor("v", [BPC, LV, D], f32, kind="ExternalInput").ap()
    o_d = nc.dram_tensor("o", [BPC, LQ, D], f32, kind="ExternalOutput").ap()

    with tile.TileContext(nc) as tc, ExitStack() as ctx:
        const = ctx.enter_context(tc.tile_pool(name="const", bufs=1))
        big = ctx.enter_context(tc.tile_pool(name="big", bufs=1))
        stage = ctx.enter_context(tc.tile_pool(name="stage", bufs=3))
        qtp = ctx.enter_context(tc.tile_pool(name="qtp", bufs=2))
        pp = ctx.enter_context(tc.tile_pool(name="pp", bufs=2))
        ptp = ctx.enter_context(tc.tile_pool(name="ptp", bufs=2))
        outp = ctx.enter_context(tc.tile_pool(name="outp", bufs=2))
        statp = ctx.enter_context(tc.tile_pool(name="statp", bufs=2))
        psum = ctx.enter_context(tc.tile_pool(name="psum", bufs=1, space="PSUM"))
        psum_t = ctx.enter_context(tc.tile_pool(name="psum_t", bufs=2, space="PSUM"))

        ident = const.tile([P, P], f32)
        make_identity(nc, ident)

        for b in range(BPC):
            VT = big.tile([P, ND, LV], f32r, tag="VT")
            Vn = big.tile([P, NVT, D], f32r, tag="Vn")
            for j in range(NVT):
                vst = stage.tile([P, D], f32, tag="stage")
                nc.sync.dma_start(vst, v_d[b, j * P : (j + 1) * P, :])
                nc.vector.tensor_copy(Vn[:, j, :], vst)
                for k in range(ND):
                    pst = psum_t.tile([P, P], f32, tag="pst")
                    nc.tensor.transpose(pst, vst[:, k * P : (k + 1) * P], ident)
                    nc.scalar.copy(VT[:, k, j * P : (j + 1) * P], pst)

            for qi in range(NQT):
                qst = stage.tile([P, D], f32, tag="stage")
                nc.sync.dma_start(qst, q_d[b, qi * P : (qi + 1) * P, :])
                QT = qtp.tile([P, ND, P], f32r, tag="QT")
                for k in range(ND):
                    pst = psum_t.tile([P, P], f32, tag="pst")
                    nc.tensor.transpose(pst, qst[:, k * P : (k + 1) * P], ident)
                    nc.scalar.copy(QT[:, k, :], pst)

                psS = psum.tile([P, LV], f32, tag="psS")
                stats = statp.tile([P, NCH], f32, tag="stats")
                for n in range(NCH):
                    sl = slice(n * VCH, (n + 1) * VCH)
                    for k in range(ND):
                        nc.tensor.matmul(
                            psS[:, sl],
                            QT[:, k, :],
                            VT[:, k, sl],
                            start=(k == 0),
                            stop=(k == ND - 1),
                        )
                    nc.vector.reduce_max(stats[:, n : n + 1], psS[:, sl], axis=AX)

                negmax = statp.tile([P, 1], f32, tag="negmax")
                nc.vector.reduce_max(negmax, stats, axis=AX, negate=True)
                sums = statp.tile([P, NCH], f32, tag="sums")
                Pt = pp.tile([P, LV], f32, tag="P")
                for n in range(NCH):
                    sl = slice(n * VCH, (n + 1) * VCH)
                    nc.scalar.activation(
                        Pt[:, sl],
                        psS[:, sl],
                        EXP,
                        bias=negmax,
                        accum_out=sums[:, n : n + 1],
                    )
                rowsum = statp.tile([P, 1], f32, tag="rowsum")
                nc.vector.reduce_sum(rowsum, sums, axis=AX)
                rinv = statp.tile([P, 1], f32, tag="rinv")
                nc.vector.reciprocal(rinv, rowsum)

                PT = ptp.tile([P, NVT, P], f32r, tag="PT")
                for j in range(NVT):
                    pst = psum_t.tile([P, P], f32, tag="pst")
                    nc.tensor.transpose(pst, Pt[:, j * P : (j + 1) * P], ident)
                    nc.vector.tensor_copy(PT[:, j, :], pst)

                psO = psum.tile([P, D], f32, tag="psO")
                out_sb = outp.tile([P, D], f32, tag="out")
                for dch in range(NDCH):
                    sl = slice(dch * DCH, (dch + 1) * DCH)
                    for j in range(NVT):
                        nc.tensor.matmul(
                            psO[:, sl],
                            PT[:, j, :],
                            Vn[:, j, sl],
                            start=(j == 0),
                            stop=(j == NVT - 1),
                        )
                    nc.vector.tensor_scalar_mul(out_sb[:, sl], psO[:, sl], rinv)
                nc.sync.dma_start(o_d[b, qi * P : (qi + 1) * P, :], out_sb)

    nc.compile()
    return nc


_NC_CACHE = None


def _get_nc():
    global _NC_CACHE
    if _NC_CACHE is None:
        _NC_CACHE = build_nc()
    return _NC_CACHE


def kernel(query: np.ndarray, value: np.ndarray) -> np.ndarray:
    query = np.ascontiguousarray(np.asarray(query, dtype=np.float32))
    value = np.ascontiguousarray(np.asarray(value, dtype=np.float32))
    assert query.shape == (B, LQ, D) and value.shape == (B, LV, D)
    nc = _get_nc()
    in_maps = [
        {
            "q": query[i * BPC : (i + 1) * BPC],
            "v": value[i * BPC : (i + 1) * BPC],
        }
        for i in range(NCORES)
    ]
    res = run_bass_kernel_spmd(nc, in_maps, list(range(NCORES)))
    out = np.concatenate([res.results[i]["o"] for i in range(NCORES)], axis=0)
    return out
